# revision 1
# baseline (speedup 1.0000x reference)
"""Bass/Trainium2 kernel for nn_BitGatConv (GAT-style message passing).

Self-contained: takes full inputs, shards edges by destination window across
8 NeuronCores (SPMD, one program), returns the full [N, HC] output.

Algorithm (per core, rotated node ids so all cores run the same program):
  Phase A (build): h = nodes_ft @ W, att_j = nodes_ft @ (W@A2),
    att_i = nodes_ft @ (W@A1); store bf16 tables
      hj_table [N_PAD, 128]  rows = [h | att_j]
      ao_table [NSHARD+1, 128] rows = [att_i | onehot64(node mod 64)]
      (row NSHARD = sentinel: att_i = -1e4 so exp()==0 for pad edges)
  Phase B (edges): for each 128-edge bin, gather hj rows by src and ao rows
    by local tgt; s = att_i + att_j; l = max(0.2*s, s); x = exp(l);
    payload = [x*h | x]; one-hot matmul accumulates [numer | denom] into a
    per-64-node-window PSUM tile (K bins per window, K uniform).
    No segment-max subtraction: logits are bounded (~|s|<10) so exp is safe,
    and softmax is shift-free identical.
  Phase C (flush): out = numer / (denom + 1e-16) + bias.
"""

import math
import os
import sys
from contextlib import ExitStack

import numpy as np

for _p in ("/opt/trn_rl_repo",):
    if _p not in sys.path:
        sys.path.insert(0, _p)

import ml_dtypes  # noqa: E402

BF16_NP = ml_dtypes.bfloat16

# ---------------------------------------------------------------------------
# Problem constants (hardcoded per contest rules)
N_NODES = 50000
N_EDGES = 800000
IN_CH = 128
HC = 64
NEG_SLOPE = 0.2
N_CORES = 8
W_WIN = 64  # nodes per scatter window (one-hot width)
SENT_ATT = -10000.0


def _cfg(n_nodes, n_edges, n_cores=N_CORES, w=W_WIN):
    nw = math.ceil(n_nodes / w)
    npc = math.ceil(nw / n_cores)  # windows per core
    if npc % 2 == 1:
        npc += 1  # need even (flush in pairs)
    n_pad = n_cores * npc * w
    nshard = npc * w
    # group_nw: windows per gather-group (batch for gathers/DVE)
    group_nw = 1
    for cand in (7, 6, 5, 4, 8, 3, 2):
        if npc % cand == 0:
            group_nw = cand
            break
    return dict(
        N=n_nodes, E=n_edges, NC=n_cores, W=w, NPC=npc,
        N_PAD=n_pad, NSHARD=nshard, GROUP_NW=group_nw,
        T_TILES=n_pad // 128, SHARD_TILES=nshard // 128,
    )


def _prep(inputs, cfg):
    """Host-side preprocessing: shard + pad + index building (numpy only)."""
    N, E, NC, W = cfg["N"], cfg["E"], cfg["NC"], cfg["W"]
    NPC, N_PAD, NSHARD = cfg["NPC"], cfg["N_PAD"], cfg["NSHARD"]

    nodes_ft = np.asarray(inputs["nodes_ft"], dtype=np.float32)
    adj = np.asarray(inputs["adj_list"])
    weight = np.asarray(inputs["weight"], dtype=np.float32)
    a1 = np.asarray(inputs["att_layer_1"], dtype=np.float32)
    a2 = np.asarray(inputs["att_layer_2"], dtype=np.float32)
    bias = np.asarray(inputs["bias"], dtype=np.float32)

    tgt = adj[0].astype(np.int64)
    src = adj[1].astype(np.int64)

    win = tgt // W
    core = win // NPC
    wloc = win % NPC
    GW = cfg["GROUP_NW"]
    HL = N_PAD // 2  # hj table split point (int16 index reach)

    src_rot = (src - core * NSHARD) % N_PAD
    half = (src_rot >= HL).astype(np.int64)  # 0 = lo table, 1 = hi table

    grp = win * 2 + half
    cnt2 = np.bincount(grp, minlength=NC * NPC * 2)
    KL = max(1, int(math.ceil(cnt2[0::2].max() / 128.0)))
    KH = max(1, int(math.ceil(cnt2[1::2].max() / 128.0)))
    K = KL + KH
    B = NPC * K  # bins per core
    NB = GW * K  # bins per gather group
    ngroups = NPC // GW

    order = np.argsort(grp, kind="stable")
    starts = np.zeros(NC * NPC * 2 + 1, dtype=np.int64)
    starts[1:] = np.cumsum(cnt2)
    rank = np.arange(E, dtype=np.int64) - starts[grp[order]]

    eo = order
    c_e = core[eo]
    wl = wloc[eo]
    g_e = wl // GW
    wlg = wl % GW
    h_e = half[eo]
    j_e = rank // 128
    p_e = rank % 128
    # bin index within core: group-major, [GW windows' lo bins | GW hi bins]
    b_e = g_e * NB + np.where(
        h_e == 0, wlg * KL + j_e, GW * KL + wlg * KH + j_e)

    # int16 idx streams in dma_gather wrapped layout (idx i -> [i%16, i//16])
    def wrap16(stream2d):
        # stream2d: [NC, L] -> [NC, 128, L//16]
        ncc, L = stream2d.shape
        w = stream2d.reshape(ncc, L // 16, 16).transpose(0, 2, 1)
        return np.ascontiguousarray(np.tile(w, (1, 8, 1)))

    ao_s = np.full((NC, B * 128), NSHARD, dtype=np.int16)
    ao_s[c_e, b_e * 128 + p_e] = (tgt[eo] - c_e * NSHARD).astype(np.int16)

    # lo/hi bin serial numbers within core (for the per-half gather streams)
    lob_e = g_e * (GW * KL) + wlg * KL + j_e
    hib_e = g_e * (GW * KH) + wlg * KH + j_e
    lo_s = np.zeros((NC, NPC * KL * 128), dtype=np.int16)
    hi_s = np.zeros((NC, NPC * KH * 128), dtype=np.int16)
    m0 = h_e == 0
    lo_s[c_e[m0], lob_e[m0] * 128 + p_e[m0]] = src_rot[eo][m0].astype(np.int16)
    m1 = ~m0
    hi_s[c_e[m1], hib_e[m1] * 128 + p_e[m1]] = (
        src_rot[eo][m1] - HL).astype(np.int16)

    ao_idx = wrap16(ao_s)
    lo_idx = wrap16(lo_s)
    hi_idx = wrap16(hi_s)

    # rotated, transposed, padded node features (bf16)
    base = np.zeros((IN_CH, N_PAD), dtype=np.float32)
    base[:, :N] = nodes_ft.T

    wh = weight.astype(BF16_NP)
    wi = (weight @ a1).astype(BF16_NP)
    wj = (weight @ a2).astype(BF16_NP)

    oh = np.zeros((NSHARD + 1, HC), dtype=np.float32)
    oh[np.arange(NSHARD), np.arange(NSHARD) % W] = 1.0
    # wide windows (W < HC unused cols stay 0); sentinel points at slot 0
    oh[NSHARD, 0] = 1.0
    oh = oh.astype(BF16_NP)

    sent_row = np.full((1, HC), SENT_ATT, dtype=np.float32).astype(BF16_NP)

    npair = NPC // 2
    bias_full = np.tile(bias[None, :], (128, npair)).astype(np.float32)

    in_maps = []
    for c in range(NC):
        nftT = np.ascontiguousarray(np.roll(base, -c * NSHARD, axis=1))
        in_maps.append({
            "nodes_ftT": nftT.astype(BF16_NP),
            "wh": wh, "wi": wi, "wj": wj,
            "onehot_const": oh,
            "sent_row": sent_row,
            "lo_idx": lo_idx[c],
            "hi_idx": hi_idx[c],
            "ao_idx": ao_idx[c],
            "bias_bc": bias_full,
        })
    meta = dict(K=K, KL=KL, KH=KH, B=B)
    return in_maps, meta


def _build_program(cfg, K, KL, KH, debug_dump=False, phase_limit="full",
                   repeat=1):
    import concourse.bacc as bacc
    import concourse.bass as bass
    import concourse.mybir as mybir
    import concourse.tile as tile

    BF16 = mybir.dt.bfloat16
    F32 = mybir.dt.float32
    I16 = mybir.dt.int16
    ALU = mybir.AluOpType
    ACT = mybir.ActivationFunctionType

    NPC, N_PAD, NSHARD = cfg["NPC"], cfg["N_PAD"], cfg["NSHARD"]
    T_TILES, SHARD_TILES = cfg["T_TILES"], cfg["SHARD_TILES"]
    GROUP_NW = cfg["GROUP_NW"]
    assert K == KL + KH
    B = NPC * K
    NB = GROUP_NW * K          # bins per gather group
    NBL = GROUP_NW * KL        # lo bins per group
    NBH = GROUP_NW * KH
    NGROUPS = NPC // GROUP_NW
    NPAIR = NPC // 2
    HL = N_PAD // 2

    nc = bacc.Bacc("TRN2", target_bir_lowering=False, debug=False,
                   num_swdge_queues=4)

    nodes_ftT = nc.dram_tensor("nodes_ftT", [IN_CH, N_PAD], BF16, kind="ExternalInput")
    wh_d = nc.dram_tensor("wh", [IN_CH, HC], BF16, kind="ExternalInput")
    wi_d = nc.dram_tensor("wi", [IN_CH, HC], BF16, kind="ExternalInput")
    wj_d = nc.dram_tensor("wj", [IN_CH, HC], BF16, kind="ExternalInput")
    oh_d = nc.dram_tensor("onehot_const", [NSHARD + 1, HC], BF16, kind="ExternalInput")
    sent_d = nc.dram_tensor("sent_row", [1, HC], BF16, kind="ExternalInput")
    loidx_d = nc.dram_tensor("lo_idx", [128, NPC * KL * 8], I16, kind="ExternalInput")
    hiidx_d = nc.dram_tensor("hi_idx", [128, NPC * KH * 8], I16, kind="ExternalInput")
    aoidx_d = nc.dram_tensor("ao_idx", [128, NPC * K * 8], I16, kind="ExternalInput")
    bias_d = nc.dram_tensor("bias_bc", [128, NPAIR * HC], F32, kind="ExternalInput")
    out_d = nc.dram_tensor("out", [NSHARD, HC], F32, kind="ExternalOutput")

    hj_table = nc.dram_tensor("hj_table", [N_PAD, 2 * HC], BF16, kind="Internal")
    ao_table = nc.dram_tensor("ao_table", [NSHARD + 1, 2 * HC], BF16, kind="Internal")

    do_build = phase_limit != "noop"
    do_gather = phase_limit in ("gather", "nomm", "full")
    do_dve = phase_limit in ("nomm", "full")
    do_mm = phase_limit == "full"

    with tile.TileContext(nc) as tc, ExitStack() as ctx:
        const_pool = ctx.enter_context(tc.tile_pool(name="const", bufs=1))
        b_in = ctx.enter_context(tc.tile_pool(name="b_in", bufs=4))
        b_ps = ctx.enter_context(tc.tile_pool(name="b_ps", bufs=2, space="PSUM"))
        b_st = ctx.enter_context(tc.tile_pool(name="b_st", bufs=4))
        idx_pool = ctx.enter_context(tc.tile_pool(name="idx", bufs=4))
        g_pool = ctx.enter_context(tc.tile_pool(name="gp", bufs=2))
        ao_pool = ctx.enter_context(tc.tile_pool(name="aop", bufs=2))
        s_pool = ctx.enter_context(tc.tile_pool(name="sp", bufs=2))
        mm_ps = ctx.enter_context(tc.tile_pool(name="mmps", bufs=4, space="PSUM"))
        fl_pool = ctx.enter_context(tc.tile_pool(name="fl", bufs=1))

        wh_sb = const_pool.tile([IN_CH, HC], BF16)
        nc.sync.dma_start(wh_sb[:], wh_d[:])
        wi_sb = const_pool.tile([IN_CH, HC], BF16)
        nc.sync.dma_start(wi_sb[:], wi_d[:])
        wj_sb = const_pool.tile([IN_CH, HC], BF16)
        nc.sync.dma_start(wj_sb[:], wj_d[:])
        bias_sb = const_pool.tile([128, NPAIR * HC], F32)
        nc.sync.dma_start(bias_sb[:], bias_d[:])

        # constant halves of ao_table (DRAM->DRAM)
        nc.sync.dma_start(ao_table[:, HC:2 * HC], oh_d[:])
        nc.sync.dma_start(ao_table[NSHARD:NSHARD + 1, 0:HC], sent_d[:])

        def emit_once(rep):
            # ---- Phase A: build tables (replicated on every core)
            # two node-tiles per iteration: batched DMAs, alternating HWDGE
            # engines (sync / scalar are separate HW-DGE rings)
            for t2 in range(T_TILES // 2 if do_build else 0):
                t = 2 * t2
                dmae = nc.sync if t2 % 2 == 0 else nc.scalar
                nf = b_in.tile([128, 2, 128], BF16, name=f"nf")
                dmae.dma_start(
                    nf[:].rearrange("p a b -> p (a b)"),
                    nodes_ftT[:, 128 * t:128 * (t + 2)])
                ps = b_ps.tile([128, 2, 2 * HC], F32, name=f"bps")
                for u in range(2):
                    nc.tensor.matmul(ps[:, u, 0:HC], nf[:, u, :], wh_sb[:],
                                     start=(u == 0), stop=False)
                    nc.tensor.matmul(ps[:, u, HC:2 * HC], nf[:, u, :], wj_sb[:],
                                     start=False, stop=(u == 1))
                st = b_st.tile([128, 2, 2 * HC], BF16, name=f"bst")
                if t2 % 2 == 0:
                    nc.vector.tensor_copy(st[:], ps[:])
                else:
                    nc.scalar.copy(st[:], ps[:])
                dmae.dma_start(
                    hj_table[128 * t:128 * (t + 2), :].rearrange(
                        "(a p) b -> p a b", p=128),
                    st[:])
            # att_i shard tiles (first SHARD_TILES node-tiles, done separately
            # so hj batching stays uniform)
            for t in range(SHARD_TILES if do_build else 0):
                nf2 = b_in.tile([128, 128], BF16, tag="nf2", name="nf2")
                dmae = nc.scalar if t % 2 == 0 else nc.sync
                dmae.dma_start(nf2[:], nodes_ftT[:, 128 * t:128 * (t + 1)])
                ps2 = b_ps.tile([128, HC], F32, tag="bps2", name="bps2")
                nc.tensor.matmul(ps2[:], nf2[:], wi_sb[:], start=True, stop=True)
                sa = b_st.tile([128, HC], BF16, tag="sa", name="sa")
                if t % 2 == 0:
                    nc.scalar.copy(sa[:], ps2[:])
                else:
                    nc.vector.tensor_copy(sa[:], ps2[:])
                dmae.dma_start(ao_table[128 * t:128 * (t + 1), 0:HC], sa[:])

            if int(os.environ.get("GAT_BARRIER", "0")):
                tc.strict_bb_all_engine_barrier()

            # ---- Phase B: edge processing
            stage_n = fl_pool.tile([128, NPAIR * HC], F32, tag="sn", name="sn")
            stage_d = fl_pool.tile([128, NPAIR * HC], F32, tag="sd", name="sd")

            pair_tiles = {}
            last_G = last_AO = None
            for g in range(NGROUPS if do_gather else 0):
                sl = idx_pool.tile([128, NBL * 8], I16, tag="sl", name="sl")
                nc.sync.dma_start(sl[:], loidx_d[:, g * NBL * 8:(g + 1) * NBL * 8])
                sh = idx_pool.tile([128, NBH * 8], I16, tag="sh", name="sh")
                nc.sync.dma_start(sh[:], hiidx_d[:, g * NBH * 8:(g + 1) * NBH * 8])
                ai = idx_pool.tile([128, NB * 8], I16, tag="ai", name="ai")
                nc.sync.dma_start(ai[:], aoidx_d[:, g * NB * 8:(g + 1) * NB * 8])

                G = g_pool.tile([128, NB, 2 * HC], BF16, tag="G", name="G")
                AOt = ao_pool.tile([128, NB, 2 * HC], BF16, tag="AO", name="AOt")
                qn = 0

                def chunked_gather(out_tile, table_ap, idx_tile, nbins, parts):
                    nonlocal qn
                    cuts = [nbins * i // parts for i in range(parts + 1)]
                    for a, b2 in zip(cuts[:-1], cuts[1:]):
                        if a == b2:
                            continue
                        nc.gpsimd.dma_gather(
                            out_ap=out_tile[:, a:b2, :], in_ap=table_ap,
                            idxs_ap=idx_tile[:, a * 8:b2 * 8],
                            num_idxs=(b2 - a) * 128,
                            num_idxs_reg=(b2 - a) * 128,
                            elem_size=2 * HC, queue_num=qn % 4,
                            single_packet=False,
                        )
                        qn += 1

                chunked_gather(G[:, 0:NBL, :].rearrange("p a b -> p a b"),
                               hj_table[0:HL, :], sl, NBL, 2)
                chunked_gather(G[:, NBL:NB, :].rearrange("p a b -> p a b"),
                               hj_table[HL:N_PAD, :], sh, NBH, 2)
                chunked_gather(AOt[:], ao_table[:], ai, NB, 4)
                last_G, last_AO = G, AOt

                if not do_dve:
                    continue
                S = s_pool.tile([128, NB, HC], BF16, tag="S", name="S")
                # s = att_j + att_i
                nc.vector.tensor_tensor(
                    out=S[:], in0=G[:, :, HC:2 * HC], in1=AOt[:, :, 0:HC], op=ALU.add)
                # l = max(0.2*s, s)  (leaky relu)
                nc.vector.scalar_tensor_tensor(
                    out=S[:], in0=S[:], scalar=NEG_SLOPE, in1=S[:],
                    op0=ALU.mult, op1=ALU.max)
                # x = exp(l) -> overwrite att_j half of G
                nc.scalar.activation(G[:, :, HC:2 * HC], S[:], ACT.Exp)
                # y = h * x -> overwrite h half of G
                nc.vector.tensor_tensor(
                    out=G[:, :, 0:HC], in0=G[:, :, 0:HC], in1=G[:, :, HC:2 * HC],
                    op=ALU.mult)

                for bl in range(NB if do_mm else 0):
                    if bl < NBL:
                        w = g * GROUP_NW + bl // KL
                        j = bl % KL
                    else:
                        l2 = bl - NBL
                        w = g * GROUP_NW + l2 // KH
                        j = KL + l2 % KH
                    pr, half = w // 2, w % 2
                    if j == 0 and half == 0:
                        pair_tiles[pr] = mm_ps.tile(
                            [128, 2 * HC], F32, tag="pp", name=f"pp{pr}")
                    ps_t = pair_tiles[pr]
                    nc.tensor.matmul(
                        ps_t[HC * half:HC * half + HC, :],
                        AOt[:, bl, HC:2 * HC],
                        G[:, bl, :],
                        start=(j == 0), stop=(j == K - 1),
                        tile_position=(0, HC * half),
                        skip_group_check=True,
                    )
                    if j == K - 1 and half == 1:
                        nc.vector.tensor_copy(
                            stage_n[:, HC * pr:HC * (pr + 1)], ps_t[:, 0:HC])
                        nc.vector.tensor_copy(
                            stage_d[:, HC * pr:HC * (pr + 1)], ps_t[:, HC:2 * HC])
                        del pair_tiles[pr]

            # ---- Phase C: out = numer / (denom + eps) + bias
            if not do_mm:
                nc.vector.memset(stage_n[:], 0.0)
                nc.vector.memset(stage_d[:], 1.0)
            nc.vector.tensor_scalar_add(stage_d[:], stage_d[:], 1e-16)
            lnd = fl_pool.tile([128, NPAIR * HC], F32, tag="lnd", name="lnd")
            nc.scalar.activation(lnd[:], stage_d[:], ACT.Ln)
            nc.scalar.activation(lnd[:], lnd[:], ACT.Exp, scale=-1.0)
            nc.vector.tensor_tensor(out=stage_n[:], in0=stage_n[:], in1=lnd[:],
                                    op=ALU.mult)
            nc.vector.tensor_tensor(out=stage_n[:], in0=stage_n[:], in1=bias_sb[:],
                                    op=ALU.add)

            out_view = out_d[:].rearrange("(pr p) c -> p pr c", p=128)
            st_view = stage_n[:].rearrange("p (pr c) -> p pr c", c=HC)
            nc.sync.dma_start(out_view, st_view)
            return last_G, last_AO, stage_d

        for rep in range(repeat):
            last_G, last_AO, stage_d = emit_once(rep)
            if repeat > 1:
                tc.strict_bb_all_engine_barrier()

        if debug_dump:
            dump_hj = nc.dram_tensor("dump_hj", [N_PAD, 2 * HC], BF16,
                                     kind="ExternalOutput")
            dump_ao = nc.dram_tensor("dump_ao", [NSHARD + 1, 2 * HC], BF16,
                                     kind="ExternalOutput")
            dump_sd = nc.dram_tensor("dump_sd", [128, NPAIR * HC], F32,
                                     kind="ExternalOutput")
            dump_g = nc.dram_tensor("dump_g", [128, NB * 2 * HC], BF16,
                                    kind="ExternalOutput")
            dump_aot = nc.dram_tensor("dump_aot", [128, NB * 2 * HC], BF16,
                                      kind="ExternalOutput")
            tc.strict_bb_all_engine_barrier()
            nc.sync.dma_start(dump_hj[:], hj_table[:])
            nc.sync.dma_start(dump_ao[:], ao_table[:])
            nc.sync.dma_start(dump_sd[:], stage_d[:])
            nc.sync.dma_start(dump_g[:], last_G[:].rearrange("p a b -> p (a b)"))
            nc.sync.dma_start(dump_aot[:], last_AO[:].rearrange("p a b -> p (a b)"))

    nc.compile()
    return nc


def kernel(**inputs):
    cfg = _cfg(N_NODES, N_EDGES)
    in_maps, meta = _prep(inputs, cfg)
    nc = _build_program(cfg, meta["K"], meta["KL"], meta["KH"])

    from concourse import bass_utils
    res = bass_utils.run_bass_kernel_spmd(
        nc, in_maps, core_ids=list(range(cfg["NC"])),
        trace=bool(int(os.environ.get("GAT_TRACE", "0"))),
    )
    kernel.last_result = res  # stash for test harness (exec_time_ns etc.)
    kernel.last_ctx = (nc, in_maps, cfg)

    NSHARD = cfg["NSHARD"]
    out_full = np.zeros((cfg["NC"] * NSHARD, HC), dtype=np.float32)
    for c in range(cfg["NC"]):
        out_full[c * NSHARD:(c + 1) * NSHARD] = res.results[c]["out"]
    return out_full[:cfg["N"]]



# revision 3
# speedup vs baseline: 1.0012x; 1.0012x over previous
"""Bass/Trainium2 kernel for nn_BitGatConv (GAT-style message passing).

Self-contained: takes full inputs, shards edges by destination window across
8 NeuronCores (SPMD, one program), returns the full [N, HC] output.

Algorithm (per core, rotated node ids so all cores run the same program):
  Phase A (build): h = nodes_ft @ W, att_j = nodes_ft @ (W@A2),
    att_i = nodes_ft @ (W@A1); store bf16 tables
      hj_table [N_PAD, 128]  rows = [h | att_j]
      ao_table [NSHARD+1, 128] rows = [att_i | onehot64(node mod 64)]
      (row NSHARD = sentinel: att_i = -1e4 so exp()==0 for pad edges)
  Phase B (edges): for each 128-edge bin, gather hj rows by src and ao rows
    by local tgt; s = att_i + att_j; l = max(0.2*s, s); x = exp(l);
    payload = [x*h | x]; one-hot matmul accumulates [numer | denom] into a
    per-64-node-window PSUM tile (K bins per window, K uniform).
    No segment-max subtraction: logits are bounded (~|s|<10) so exp is safe,
    and softmax is shift-free identical.
  Phase C (flush): out = numer / (denom + 1e-16) + bias.
"""

import math
import os
import sys
from contextlib import ExitStack

import numpy as np

for _p in ("/opt/trn_rl_repo",):
    if _p not in sys.path:
        sys.path.insert(0, _p)

import ml_dtypes  # noqa: E402

BF16_NP = ml_dtypes.bfloat16

# ---------------------------------------------------------------------------
# Problem constants (hardcoded per contest rules)
N_NODES = 50000
N_EDGES = 800000
IN_CH = 128
HC = 64
NEG_SLOPE = 0.2
N_CORES = 8
W_WIN = 64  # nodes per scatter window (one-hot width)
SENT_ATT = -10000.0


def _cfg(n_nodes, n_edges, n_cores=N_CORES, w=W_WIN):
    nw = math.ceil(n_nodes / w)
    npc = math.ceil(nw / n_cores)  # windows per core
    if npc % 2 == 1:
        npc += 1  # need even (flush in pairs)
    n_pad = n_cores * npc * w
    nshard = npc * w
    # group_nw: windows per gather-group (batch for gathers/DVE)
    group_nw = 1
    for cand in (7, 6, 5, 4, 8, 3, 2):
        if npc % cand == 0:
            group_nw = cand
            break
    return dict(
        N=n_nodes, E=n_edges, NC=n_cores, W=w, NPC=npc,
        N_PAD=n_pad, NSHARD=nshard, GROUP_NW=group_nw,
        T_TILES=n_pad // 128, SHARD_TILES=nshard // 128,
    )


def _prep(inputs, cfg):
    """Host-side preprocessing: shard + pad + index building (numpy only)."""
    N, E, NC, W = cfg["N"], cfg["E"], cfg["NC"], cfg["W"]
    NPC, N_PAD, NSHARD = cfg["NPC"], cfg["N_PAD"], cfg["NSHARD"]

    nodes_ft = np.asarray(inputs["nodes_ft"], dtype=np.float32)
    adj = np.asarray(inputs["adj_list"])
    weight = np.asarray(inputs["weight"], dtype=np.float32)
    a1 = np.asarray(inputs["att_layer_1"], dtype=np.float32)
    a2 = np.asarray(inputs["att_layer_2"], dtype=np.float32)
    bias = np.asarray(inputs["bias"], dtype=np.float32)

    tgt = adj[0].astype(np.int64)
    src = adj[1].astype(np.int64)

    win = tgt // W
    core = win // NPC
    wloc = win % NPC
    GW = cfg["GROUP_NW"]
    HL = N_PAD // 2  # hj table split point (int16 index reach)

    src_rot = (src - core * NSHARD) % N_PAD
    half = (src_rot >= HL).astype(np.int64)  # 0 = lo table, 1 = hi table

    grp = win * 2 + half
    cnt2 = np.bincount(grp, minlength=NC * NPC * 2)
    KL = max(1, int(math.ceil(cnt2[0::2].max() / 128.0)))
    KH = max(1, int(math.ceil(cnt2[1::2].max() / 128.0)))
    K = KL + KH
    B = NPC * K  # bins per core
    NB = GW * K  # bins per gather group
    ngroups = NPC // GW

    order = np.argsort(grp, kind="stable")
    starts = np.zeros(NC * NPC * 2 + 1, dtype=np.int64)
    starts[1:] = np.cumsum(cnt2)
    rank = np.arange(E, dtype=np.int64) - starts[grp[order]]

    eo = order
    c_e = core[eo]
    wl = wloc[eo]
    g_e = wl // GW
    wlg = wl % GW
    h_e = half[eo]
    j_e = rank // 128
    p_e = rank % 128
    # bin index within core: group-major, [GW windows' lo bins | GW hi bins]
    b_e = g_e * NB + np.where(
        h_e == 0, wlg * KL + j_e, GW * KL + wlg * KH + j_e)

    # int16 idx streams in dma_gather wrapped layout (idx i -> [i%16, i//16])
    def wrap16(stream2d):
        # stream2d: [NC, L] -> [NC, 128, L//16]
        ncc, L = stream2d.shape
        w = stream2d.reshape(ncc, L // 16, 16).transpose(0, 2, 1)
        return np.ascontiguousarray(np.tile(w, (1, 8, 1)))

    ao_s = np.full((NC, B * 128), NSHARD, dtype=np.int16)
    ao_s[c_e, b_e * 128 + p_e] = (tgt[eo] - c_e * NSHARD).astype(np.int16)

    # lo/hi bin serial numbers within core (for the per-half gather streams)
    lob_e = g_e * (GW * KL) + wlg * KL + j_e
    hib_e = g_e * (GW * KH) + wlg * KH + j_e
    lo_s = np.zeros((NC, NPC * KL * 128), dtype=np.int16)
    hi_s = np.zeros((NC, NPC * KH * 128), dtype=np.int16)
    m0 = h_e == 0
    lo_s[c_e[m0], lob_e[m0] * 128 + p_e[m0]] = src_rot[eo][m0].astype(np.int16)
    m1 = ~m0
    hi_s[c_e[m1], hib_e[m1] * 128 + p_e[m1]] = (
        src_rot[eo][m1] - HL).astype(np.int16)

    ao_idx = wrap16(ao_s)
    lo_idx = wrap16(lo_s)
    hi_idx = wrap16(hi_s)

    # rotated, transposed, padded node features (bf16)
    base = np.zeros((IN_CH, N_PAD), dtype=np.float32)
    base[:, :N] = nodes_ft.T

    wh = weight.astype(BF16_NP)
    wi = (weight @ a1).astype(BF16_NP)
    wj = (weight @ a2).astype(BF16_NP)

    oh = np.zeros((NSHARD + 1, HC), dtype=np.float32)
    oh[np.arange(NSHARD), np.arange(NSHARD) % W] = 1.0
    # wide windows (W < HC unused cols stay 0); sentinel points at slot 0
    oh[NSHARD, 0] = 1.0
    oh = oh.astype(BF16_NP)

    sent_row = np.full((1, HC), SENT_ATT, dtype=np.float32).astype(BF16_NP)

    npair = NPC // 2
    bias_full = np.tile(bias[None, :], (128, npair)).astype(np.float32)

    in_maps = []
    for c in range(NC):
        nftT = np.ascontiguousarray(np.roll(base, -c * NSHARD, axis=1))
        in_maps.append({
            "nodes_ftT": nftT.astype(BF16_NP),
            "wh": wh, "wi": wi, "wj": wj,
            "onehot_const": oh,
            "sent_row": sent_row,
            "lo_idx": lo_idx[c],
            "hi_idx": hi_idx[c],
            "ao_idx": ao_idx[c],
            "bias_bc": bias_full,
        })
    meta = dict(K=K, KL=KL, KH=KH, B=B)
    return in_maps, meta


def _build_program(cfg, K, KL, KH, debug_dump=False, phase_limit="full",
                   repeat=1):
    import concourse.bacc as bacc
    import concourse.bass as bass
    import concourse.mybir as mybir
    import concourse.tile as tile

    BF16 = mybir.dt.bfloat16
    F32 = mybir.dt.float32
    I16 = mybir.dt.int16
    ALU = mybir.AluOpType
    ACT = mybir.ActivationFunctionType

    NPC, N_PAD, NSHARD = cfg["NPC"], cfg["N_PAD"], cfg["NSHARD"]
    T_TILES, SHARD_TILES = cfg["T_TILES"], cfg["SHARD_TILES"]
    GROUP_NW = cfg["GROUP_NW"]
    assert K == KL + KH
    B = NPC * K
    NB = GROUP_NW * K          # bins per gather group
    NBL = GROUP_NW * KL        # lo bins per group
    NBH = GROUP_NW * KH
    NGROUPS = NPC // GROUP_NW
    NPAIR = NPC // 2
    HL = N_PAD // 2

    nc = bacc.Bacc("TRN2", target_bir_lowering=False, debug=False,
                   num_swdge_queues=4)

    nodes_ftT = nc.dram_tensor("nodes_ftT", [IN_CH, N_PAD], BF16, kind="ExternalInput")
    wh_d = nc.dram_tensor("wh", [IN_CH, HC], BF16, kind="ExternalInput")
    wi_d = nc.dram_tensor("wi", [IN_CH, HC], BF16, kind="ExternalInput")
    wj_d = nc.dram_tensor("wj", [IN_CH, HC], BF16, kind="ExternalInput")
    oh_d = nc.dram_tensor("onehot_const", [NSHARD + 1, HC], BF16, kind="ExternalInput")
    sent_d = nc.dram_tensor("sent_row", [1, HC], BF16, kind="ExternalInput")
    loidx_d = nc.dram_tensor("lo_idx", [128, NPC * KL * 8], I16, kind="ExternalInput")
    hiidx_d = nc.dram_tensor("hi_idx", [128, NPC * KH * 8], I16, kind="ExternalInput")
    aoidx_d = nc.dram_tensor("ao_idx", [128, NPC * K * 8], I16, kind="ExternalInput")
    bias_d = nc.dram_tensor("bias_bc", [128, NPAIR * HC], F32, kind="ExternalInput")
    out_d = nc.dram_tensor("out", [NSHARD, HC], F32, kind="ExternalOutput")

    hj_table = nc.dram_tensor("hj_table", [N_PAD, 2 * HC], BF16, kind="Internal")
    ao_table = nc.dram_tensor("ao_table", [NSHARD + 1, 2 * HC], BF16, kind="Internal")

    do_build = phase_limit != "noop"
    do_gather = phase_limit in ("gather", "nomm", "full")
    do_dve = phase_limit in ("nomm", "full")
    do_mm = phase_limit == "full"

    # bench knobs (defaults preserve normal behavior)
    nq = int(os.environ.get("GAT_NQ", "4"))
    skip_ao = bool(int(os.environ.get("GAT_SKIP_AO", "0")))
    skip_hj = bool(int(os.environ.get("GAT_SKIP_HJ", "0")))
    hj_chunks = int(os.environ.get("GAT_HJ_CHUNKS", "2"))
    ao_chunks = int(os.environ.get("GAT_AO_CHUNKS", "4"))

    with tile.TileContext(nc) as tc, ExitStack() as ctx:
        const_pool = ctx.enter_context(tc.tile_pool(name="const", bufs=1))
        b_in = ctx.enter_context(tc.tile_pool(name="b_in", bufs=4))
        b_ps = ctx.enter_context(tc.tile_pool(name="b_ps", bufs=2, space="PSUM"))
        b_st = ctx.enter_context(tc.tile_pool(name="b_st", bufs=4))
        idx_pool = ctx.enter_context(tc.tile_pool(name="idx", bufs=4))
        g_pool = ctx.enter_context(tc.tile_pool(name="gp", bufs=2))
        ao_pool = ctx.enter_context(tc.tile_pool(name="aop", bufs=2))
        s_pool = ctx.enter_context(tc.tile_pool(name="sp", bufs=2))
        mm_ps = ctx.enter_context(tc.tile_pool(name="mmps", bufs=4, space="PSUM"))
        fl_pool = ctx.enter_context(tc.tile_pool(name="fl", bufs=1))

        wh_sb = const_pool.tile([IN_CH, HC], BF16)
        nc.sync.dma_start(wh_sb[:], wh_d[:])
        wi_sb = const_pool.tile([IN_CH, HC], BF16)
        nc.sync.dma_start(wi_sb[:], wi_d[:])
        wj_sb = const_pool.tile([IN_CH, HC], BF16)
        nc.sync.dma_start(wj_sb[:], wj_d[:])
        bias_sb = const_pool.tile([128, NPAIR * HC], F32)
        nc.sync.dma_start(bias_sb[:], bias_d[:])

        # constant halves of ao_table (DRAM->DRAM)
        nc.sync.dma_start(ao_table[:, HC:2 * HC], oh_d[:])
        nc.sync.dma_start(ao_table[NSHARD:NSHARD + 1, 0:HC], sent_d[:])

        def emit_once(rep):
            # ---- Phase A: build tables (replicated on every core)
            # two node-tiles per iteration: batched DMAs, alternating HWDGE
            # engines (sync / scalar are separate HW-DGE rings)
            for t2 in range(T_TILES // 2 if do_build else 0):
                t = 2 * t2
                dmae = nc.sync if t2 % 2 == 0 else nc.scalar
                nf = b_in.tile([128, 2, 128], BF16, name=f"nf")
                dmae.dma_start(
                    nf[:].rearrange("p a b -> p (a b)"),
                    nodes_ftT[:, 128 * t:128 * (t + 2)])
                ps = b_ps.tile([128, 2, 2 * HC], F32, name=f"bps")
                for u in range(2):
                    nc.tensor.matmul(ps[:, u, 0:HC], nf[:, u, :], wh_sb[:],
                                     start=(u == 0), stop=False)
                    nc.tensor.matmul(ps[:, u, HC:2 * HC], nf[:, u, :], wj_sb[:],
                                     start=False, stop=(u == 1))
                st = b_st.tile([128, 2, 2 * HC], BF16, name=f"bst")
                if t2 % 2 == 0:
                    nc.vector.tensor_copy(st[:], ps[:])
                else:
                    nc.scalar.copy(st[:], ps[:])
                dmae.dma_start(
                    hj_table[128 * t:128 * (t + 2), :].rearrange(
                        "(a p) b -> p a b", p=128),
                    st[:])
            # att_i shard tiles (first SHARD_TILES node-tiles, done separately
            # so hj batching stays uniform)
            for t in range(SHARD_TILES if do_build else 0):
                nf2 = b_in.tile([128, 128], BF16, tag="nf2", name="nf2")
                dmae = nc.scalar if t % 2 == 0 else nc.sync
                dmae.dma_start(nf2[:], nodes_ftT[:, 128 * t:128 * (t + 1)])
                ps2 = b_ps.tile([128, HC], F32, tag="bps2", name="bps2")
                nc.tensor.matmul(ps2[:], nf2[:], wi_sb[:], start=True, stop=True)
                sa = b_st.tile([128, HC], BF16, tag="sa", name="sa")
                if t % 2 == 0:
                    nc.scalar.copy(sa[:], ps2[:])
                else:
                    nc.vector.tensor_copy(sa[:], ps2[:])
                dmae.dma_start(ao_table[128 * t:128 * (t + 1), 0:HC], sa[:])

            if int(os.environ.get("GAT_BARRIER", "0")):
                tc.strict_bb_all_engine_barrier()

            # ---- Phase B: edge processing
            stage_n = fl_pool.tile([128, NPAIR * HC], F32, tag="sn", name="sn")
            stage_d = fl_pool.tile([128, NPAIR * HC], F32, tag="sd", name="sd")

            pair_tiles = {}
            last_G = last_AO = None
            for g in range(NGROUPS if do_gather else 0):
                sl = idx_pool.tile([128, NBL * 8], I16, tag="sl", name="sl")
                nc.sync.dma_start(sl[:], loidx_d[:, g * NBL * 8:(g + 1) * NBL * 8])
                sh = idx_pool.tile([128, NBH * 8], I16, tag="sh", name="sh")
                nc.sync.dma_start(sh[:], hiidx_d[:, g * NBH * 8:(g + 1) * NBH * 8])
                ai = idx_pool.tile([128, NB * 8], I16, tag="ai", name="ai")
                nc.sync.dma_start(ai[:], aoidx_d[:, g * NB * 8:(g + 1) * NB * 8])

                G = g_pool.tile([128, NB, 2 * HC], BF16, tag="G", name="G")
                AOt = ao_pool.tile([128, NB, 2 * HC], BF16, tag="AO", name="AOt")
                qn = 0

                def chunked_gather(out_tile, table_ap, idx_tile, nbins, parts):
                    nonlocal qn
                    cuts = [nbins * i // parts for i in range(parts + 1)]
                    for a, b2 in zip(cuts[:-1], cuts[1:]):
                        if a == b2:
                            continue
                        nc.gpsimd.dma_gather(
                            out_ap=out_tile[:, a:b2, :], in_ap=table_ap,
                            idxs_ap=idx_tile[:, a * 8:b2 * 8],
                            num_idxs=(b2 - a) * 128,
                            num_idxs_reg=(b2 - a) * 128,
                            elem_size=2 * HC, queue_num=qn % nq,
                            single_packet=False,
                        )
                        qn += 1

                if not skip_hj:
                    chunked_gather(G[:, 0:NBL, :].rearrange("p a b -> p a b"),
                                   hj_table[0:HL, :], sl, NBL, hj_chunks)
                    chunked_gather(G[:, NBL:NB, :].rearrange("p a b -> p a b"),
                                   hj_table[HL:N_PAD, :], sh, NBH, hj_chunks)
                if not skip_ao:
                    chunked_gather(AOt[:], ao_table[:], ai, NB, ao_chunks)
                last_G, last_AO = G, AOt

                if not do_dve:
                    continue
                S = s_pool.tile([128, NB, HC], BF16, tag="S", name="S")
                # s = att_j + att_i
                nc.vector.tensor_tensor(
                    out=S[:], in0=G[:, :, HC:2 * HC], in1=AOt[:, :, 0:HC], op=ALU.add)
                # l = max(0.2*s, s)  (leaky relu)
                nc.vector.scalar_tensor_tensor(
                    out=S[:], in0=S[:], scalar=NEG_SLOPE, in1=S[:],
                    op0=ALU.mult, op1=ALU.max)
                # x = exp(l) -> overwrite att_j half of G
                nc.scalar.activation(G[:, :, HC:2 * HC], S[:], ACT.Exp)
                # y = h * x -> overwrite h half of G
                nc.vector.tensor_tensor(
                    out=G[:, :, 0:HC], in0=G[:, :, 0:HC], in1=G[:, :, HC:2 * HC],
                    op=ALU.mult)

                for bl in range(NB if do_mm else 0):
                    if bl < NBL:
                        w = g * GROUP_NW + bl // KL
                        j = bl % KL
                    else:
                        l2 = bl - NBL
                        w = g * GROUP_NW + l2 // KH
                        j = KL + l2 % KH
                    pr, half = w // 2, w % 2
                    if j == 0 and half == 0:
                        pair_tiles[pr] = mm_ps.tile(
                            [128, 2 * HC], F32, tag="pp", name=f"pp{pr}")
                    ps_t = pair_tiles[pr]
                    nc.tensor.matmul(
                        ps_t[HC * half:HC * half + HC, :],
                        AOt[:, bl, HC:2 * HC],
                        G[:, bl, :],
                        start=(j == 0), stop=(j == K - 1),
                        tile_position=(0, HC * half),
                        skip_group_check=True,
                    )
                    if j == K - 1 and half == 1:
                        nc.vector.tensor_copy(
                            stage_n[:, HC * pr:HC * (pr + 1)], ps_t[:, 0:HC])
                        nc.vector.tensor_copy(
                            stage_d[:, HC * pr:HC * (pr + 1)], ps_t[:, HC:2 * HC])
                        del pair_tiles[pr]

            # ---- Phase C: out = numer / (denom + eps) + bias
            if not do_mm:
                nc.vector.memset(stage_n[:], 0.0)
                nc.vector.memset(stage_d[:], 1.0)
            nc.vector.tensor_scalar_add(stage_d[:], stage_d[:], 1e-16)
            lnd = fl_pool.tile([128, NPAIR * HC], F32, tag="lnd", name="lnd")
            nc.scalar.activation(lnd[:], stage_d[:], ACT.Ln)
            nc.scalar.activation(lnd[:], lnd[:], ACT.Exp, scale=-1.0)
            nc.vector.tensor_tensor(out=stage_n[:], in0=stage_n[:], in1=lnd[:],
                                    op=ALU.mult)
            nc.vector.tensor_tensor(out=stage_n[:], in0=stage_n[:], in1=bias_sb[:],
                                    op=ALU.add)

            out_view = out_d[:].rearrange("(pr p) c -> p pr c", p=128)
            st_view = stage_n[:].rearrange("p (pr c) -> p pr c", c=HC)
            nc.sync.dma_start(out_view, st_view)
            return last_G, last_AO, stage_d

        for rep in range(repeat):
            last_G, last_AO, stage_d = emit_once(rep)
            if repeat > 1:
                tc.strict_bb_all_engine_barrier()

        if debug_dump:
            dump_hj = nc.dram_tensor("dump_hj", [N_PAD, 2 * HC], BF16,
                                     kind="ExternalOutput")
            dump_ao = nc.dram_tensor("dump_ao", [NSHARD + 1, 2 * HC], BF16,
                                     kind="ExternalOutput")
            dump_sd = nc.dram_tensor("dump_sd", [128, NPAIR * HC], F32,
                                     kind="ExternalOutput")
            dump_g = nc.dram_tensor("dump_g", [128, NB * 2 * HC], BF16,
                                    kind="ExternalOutput")
            dump_aot = nc.dram_tensor("dump_aot", [128, NB * 2 * HC], BF16,
                                      kind="ExternalOutput")
            tc.strict_bb_all_engine_barrier()
            nc.sync.dma_start(dump_hj[:], hj_table[:])
            nc.sync.dma_start(dump_ao[:], ao_table[:])
            nc.sync.dma_start(dump_sd[:], stage_d[:])
            nc.sync.dma_start(dump_g[:], last_G[:].rearrange("p a b -> p (a b)"))
            nc.sync.dma_start(dump_aot[:], last_AO[:].rearrange("p a b -> p (a b)"))

    nc.compile()
    return nc


def kernel(**inputs):
    cfg = _cfg(N_NODES, N_EDGES)
    in_maps, meta = _prep(inputs, cfg)
    nc = _build_program(cfg, meta["K"], meta["KL"], meta["KH"])

    from concourse import bass_utils
    res = bass_utils.run_bass_kernel_spmd(
        nc, in_maps, core_ids=list(range(cfg["NC"])),
        trace=bool(int(os.environ.get("GAT_TRACE", "0"))),
    )
    kernel.last_result = res  # stash for test harness (exec_time_ns etc.)
    kernel.last_ctx = (nc, in_maps, cfg)

    NSHARD = cfg["NSHARD"]
    out_full = np.zeros((cfg["NC"] * NSHARD, HC), dtype=np.float32)
    for c in range(cfg["NC"]):
        out_full[c * NSHARD:(c + 1) * NSHARD] = res.results[c]["out"]
    return out_full[:cfg["N"]]



# revision 4
# speedup vs baseline: 86.4836x; 86.3802x over previous
"""Bass/Trainium2 kernel for nn_BitGatConv (GAT-style message passing), V2.

Self-contained: takes full inputs, shards edges by destination window-pair
across 8 NeuronCores (SPMD, one program), returns the full [N, HC] output.

V2 vs V1: the per-edge ao_table gather (att_i + onehot, 256B/edge) is
eliminated. Instead:
  - destination windows are 128 nodes wide ("pairs"); per pair the scatter
    one-hot [128e, 128slots] is built on-device from a tiny bf16 stream of
    local target ids (tl) via an is_equal against an iota row,
  - the transposed one-hot [128slots, 128e] is built from host-packed
    bitmasks via bitwise_and + is_equal,
  - att_i lives in SBUF (built in Phase A); per-edge att_i[tgt] comes from a
    per-bin matmul  onehotT.T @ att_i_pair,
  - gather pad indices are -1 (trailing per pair-half) so the DMA rings skip
    them; gathered-garbage slots are neutralized by a G memset + zero one-hot
    columns.

Algorithm per core (rotated node ids so all cores run the same program):
  Phase A: ai_sb = nodes_ft @ (W@A1) -> SBUF;  hj tables [h | att_j] -> DRAM
    (split lo/hi for int16 gather reach).
  Phase B (per pair): gather hj rows by src; build OH/OHT; ai_pe = OHT.T@ai;
    s = ai_pe + att_j; l = max(0.2 s, s); x = exp(l); y = h*x;
    [numer|denom] += OH.T @ [y|x]  (PSUM accumulate over the pair's bins).
  Phase C: out = numer / (denom + 1e-16) + bias.
  No segment-max subtraction: logits are bounded (|s| ~< 12) so exp is safe,
  and softmax is shift-free identical.
"""

import math
import os
import sys
from contextlib import ExitStack

import numpy as np

for _p in ("/opt/trn_rl_repo",):
    if _p not in sys.path:
        sys.path.insert(0, _p)

import ml_dtypes  # noqa: E402

BF16_NP = ml_dtypes.bfloat16

# ---------------------------------------------------------------------------
# Problem constants (hardcoded per contest rules)
N_NODES = 50000
N_EDGES = 800000
IN_CH = 128
HC = 64
NEG_SLOPE = 0.2
N_CORES = 8
PW = 128  # nodes per scatter pair (one-hot width)


def _cfg(n_nodes=N_NODES, n_cores=N_CORES):
    npc = math.ceil(math.ceil(n_nodes / PW) / n_cores)  # pairs per core
    nshard = npc * PW
    n_pad = n_cores * nshard
    return dict(
        N=n_nodes, NC=n_cores, NPC=npc, NSHARD=nshard, N_PAD=n_pad,
        HL=n_pad // 2, T_TILES=n_pad // 128,
    )


def _prep(inputs, cfg):
    """Host-side preprocessing: shard + pad + index building (numpy only)."""
    NC, NPC, NSHARD, N_PAD, HL = (
        cfg["NC"], cfg["NPC"], cfg["NSHARD"], cfg["N_PAD"], cfg["HL"])
    N = cfg["N"]

    nodes_ft = np.asarray(inputs["nodes_ft"], dtype=np.float32)
    adj = np.asarray(inputs["adj_list"])
    weight = np.asarray(inputs["weight"], dtype=np.float32)
    a1 = np.asarray(inputs["att_layer_1"], dtype=np.float32)
    a2 = np.asarray(inputs["att_layer_2"], dtype=np.float32)
    bias = np.asarray(inputs["bias"], dtype=np.float32)

    tgt = adj[0].astype(np.int64)
    src = adj[1].astype(np.int64)
    E = tgt.shape[0]

    pair = tgt // PW
    core = pair // NPC
    ploc = pair % NPC
    tl = (tgt % PW).astype(np.int64)

    src_rot = (src - core * NSHARD) % N_PAD
    half = (src_rot >= HL).astype(np.int64)
    idx16 = (src_rot - half * HL).astype(np.int16)

    grp = (core * NPC + ploc) * 2 + half
    cnt2 = np.bincount(grp, minlength=NC * NPC * 2)
    K_LO = max(1, int(math.ceil(cnt2[0::2].max() / 128.0)))
    K_HI = max(1, int(math.ceil(cnt2[1::2].max() / 128.0)))
    NB = K_LO + K_HI
    B = NPC * NB

    # sort each (core, pair, half) segment by source row: the gather's DMA
    # descriptors then walk ascending addresses (better DRAM locality)
    if int(os.environ.get("GAT_SRCSORT", "1")):
        order = np.lexsort((idx16, grp))
    else:
        order = np.argsort(grp, kind="stable")
    starts = np.zeros(NC * NPC * 2 + 1, dtype=np.int64)
    starts[1:] = np.cumsum(cnt2)
    rank = np.arange(E, dtype=np.int64) - starts[grp[order]]

    c_e = core[order]
    p_e = ploc[order]
    h_e = half[order]
    r_e = rank
    tl_e = tl[order]
    i_e = idx16[order]

    bl_e = np.where(h_e == 0, r_e // 128, K_LO + r_e // 128)  # bin in pair
    gb_e = p_e * NB + bl_e                                    # bin in core
    sl_e = r_e % 128                                          # slot in bin

    # gather idx streams: real idxs, then dummy row-0 up to the cross-core max
    # count for that (pair, half) -- num_idxs_reg must equal the non-negative
    # count and must be uniform across cores (SPMD one-program) -- then -1
    # trailing pads, which the DMA rings skip.
    cnt3 = cnt2.reshape(NC, NPC, 2)
    # each pair-half is gathered in two chunks (4 calls/pair, one per SWDGE
    # queue); per-chunk true counts, then cross-core max (uniform SPMD args)
    KL2 = (K_LO + 1) // 2
    KH2 = (K_HI + 1) // 2
    caps = np.array([KL2 * 128, (K_LO - KL2) * 128,
                     KH2 * 128, (K_HI - KH2) * 128])
    tc0 = np.minimum(cnt3[:, :, 0], caps[0])
    tc1 = cnt3[:, :, 0] - tc0
    tc2 = np.minimum(cnt3[:, :, 1], caps[2])
    tc3 = cnt3[:, :, 1] - tc2
    tc = np.stack([tc0, tc1, tc2, tc3], axis=-1)       # [NC, NPC, 4]
    # uniform (cross-core max, SPMD one-program) per-chunk counts; each core
    # fills [true, cnt_u) with dummy row-0 idxs, then -1 trailing pads that
    # the DMA rings skip.  (A per-core exact-count variant via GPSIMD
    # reg_load was measured slower: the 196 reg_loads cost more than the
    # ~4% dummy-descriptor saving.)
    cnt_u = np.maximum(tc.max(axis=0), 1)              # [NPC, 4]
    lo_s = np.full((NC, NPC * K_LO * 128), -1, dtype=np.int16)
    hi_s = np.full((NC, NPC * K_HI * 128), -1, dtype=np.int16)
    for c in range(NC):
        for p in range(NPC):
            blo = p * K_LO * 128
            bhi = p * K_HI * 128
            lo_s[c, blo + tc[c, p, 0]:blo + cnt_u[p, 0]] = 0
            lo_s[c, blo + caps[0] + tc[c, p, 1]:
                 blo + caps[0] + cnt_u[p, 1]] = 0
            hi_s[c, bhi + tc[c, p, 2]:bhi + cnt_u[p, 2]] = 0
            hi_s[c, bhi + caps[2] + tc[c, p, 3]:
                 bhi + caps[2] + cnt_u[p, 3]] = 0
    m0 = h_e == 0
    lo_s[c_e[m0], p_e[m0] * (K_LO * 128) + r_e[m0]] = i_e[m0]
    m1 = ~m0
    hi_s[c_e[m1], p_e[m1] * (K_HI * 128) + r_e[m1]] = i_e[m1]

    def wrap16(stream2d):
        # [NC, L] -> [NC, 128, L//16]: wrapped in 16 partitions, replicated x8
        ncc, L = stream2d.shape
        w = stream2d.reshape(ncc, L // 16, 16).transpose(0, 2, 1)
        return np.ascontiguousarray(np.tile(w, (1, 8, 1)))

    lo_idx = wrap16(lo_s)
    hi_idx = wrap16(hi_s)

    # local target ids per (slot, bin); pads 255 (matches no iota value)
    tl_bf = np.full((NC, 128, B), 255.0, dtype=np.float32)
    tl_bf[c_e, sl_e, gb_e] = tl_e.astype(np.float32)
    tl_bf = tl_bf.astype(BF16_NP)

    # transposed one-hot as packed bits: bit (k, gb*128+sl) set iff tl==k
    oht_bool = np.zeros((NC, 128, B * 128), dtype=bool)
    oht_bool[c_e, tl_e, gb_e * 128 + sl_e] = True
    bitsT = np.packbits(oht_bool, axis=-1, bitorder="little")
    bitsT = np.ascontiguousarray(bitsT).view(np.int16)  # [NC, 128, B*8]

    mask16 = np.tile((1 << np.arange(16, dtype=np.uint32)).astype(np.uint16),
                     (128, 1)).view(np.int16)
    iota128 = np.tile(np.arange(128, dtype=np.float32), (128, 1)).astype(BF16_NP)

    whj = np.concatenate([weight, weight @ a2], axis=1).astype(BF16_NP)
    wi = (weight @ a1).astype(BF16_NP)

    base = np.zeros((IN_CH, N_PAD), dtype=np.float32)
    base[:, :N] = nodes_ft.T

    bias_bc = np.tile(bias[None, :], (128, NPC)).astype(np.float32)

    in_maps = []
    for c in range(NC):
        nftT = np.ascontiguousarray(np.roll(base, -c * NSHARD, axis=1))
        in_maps.append({
            "nodes_ftT": nftT.astype(BF16_NP),
            "whj": whj, "wi": wi,
            "mask16": mask16, "iota128": iota128,
            "lo_idx": lo_idx[c], "hi_idx": hi_idx[c],
            "tl_bf": tl_bf[c], "bitsT": bitsT[c],
            "bias_bc": bias_bc,
        })
    meta = dict(K_LO=K_LO, K_HI=K_HI, NB=NB, B=B, cnt_u=cnt_u)
    return in_maps, meta


def _build_program(cfg, K_LO, K_HI, cnt_u, phase_limit="full", repeat=1):
    import concourse.bacc as bacc
    import concourse.bass as bass  # noqa: F401
    import concourse.mybir as mybir
    import concourse.tile as tile

    BF16 = mybir.dt.bfloat16
    F32 = mybir.dt.float32
    I16 = mybir.dt.int16
    ALU = mybir.AluOpType
    ACT = mybir.ActivationFunctionType

    NPC, N_PAD, NSHARD, HL = cfg["NPC"], cfg["N_PAD"], cfg["NSHARD"], cfg["HL"]
    T_TILES = cfg["T_TILES"]
    NB = K_LO + K_HI
    B = NPC * NB
    HT = T_TILES // 2  # node tiles per half table

    do_build = phase_limit != "noop"
    do_gather = phase_limit in ("gather", "nomm", "full")
    do_dve = phase_limit in ("nomm", "full")
    do_mm = phase_limit == "full"

    nc = bacc.Bacc("TRN2", target_bir_lowering=False, debug=False,
                   num_swdge_queues=4)

    nodes_ftT = nc.dram_tensor("nodes_ftT", [IN_CH, N_PAD], BF16, kind="ExternalInput")
    whj_d = nc.dram_tensor("whj", [IN_CH, 2 * HC], BF16, kind="ExternalInput")
    wi_d = nc.dram_tensor("wi", [IN_CH, HC], BF16, kind="ExternalInput")
    mask_d = nc.dram_tensor("mask16", [128, 16], I16, kind="ExternalInput")
    iota_d = nc.dram_tensor("iota128", [128, 128], BF16, kind="ExternalInput")
    loidx_d = nc.dram_tensor("lo_idx", [128, NPC * K_LO * 8], I16, kind="ExternalInput")
    hiidx_d = nc.dram_tensor("hi_idx", [128, NPC * K_HI * 8], I16, kind="ExternalInput")
    tl_d = nc.dram_tensor("tl_bf", [128, B], BF16, kind="ExternalInput")
    bits_d = nc.dram_tensor("bitsT", [128, B * 8], I16, kind="ExternalInput")
    bias_d = nc.dram_tensor("bias_bc", [128, NPC * HC], F32, kind="ExternalInput")
    out_d = nc.dram_tensor("out", [NSHARD, HC], F32, kind="ExternalOutput")

    hj_lo = nc.dram_tensor("hj_lo", [HL, 2 * HC], BF16, kind="Internal")
    hj_hi = nc.dram_tensor("hj_hi", [HL, 2 * HC], BF16, kind="Internal")

    with tile.TileContext(nc) as tc, ExitStack() as ctx:
        const_pool = ctx.enter_context(tc.tile_pool(name="const", bufs=1))
        b_in = ctx.enter_context(tc.tile_pool(name="b_in", bufs=4))
        b_ps = ctx.enter_context(tc.tile_pool(name="b_ps", bufs=2, space="PSUM"))
        b_st = ctx.enter_context(tc.tile_pool(name="b_st", bufs=4))
        ai_pool = ctx.enter_context(tc.tile_pool(name="aip", bufs=1))
        idx_pool = ctx.enter_context(tc.tile_pool(name="idx", bufs=3))
        g_pool = ctx.enter_context(tc.tile_pool(name="gp", bufs=5))
        oh_pool = ctx.enter_context(tc.tile_pool(name="ohp", bufs=5))
        oht_pool = ctx.enter_context(tc.tile_pool(name="ohtp", bufs=3))
        tmp_pool = ctx.enter_context(tc.tile_pool(name="tmpp", bufs=2))
        s_pool = ctx.enter_context(tc.tile_pool(name="sp", bufs=2))
        aipe_pool = ctx.enter_context(tc.tile_pool(name="aipe", bufs=4))
        aips_pool = ctx.enter_context(tc.tile_pool(name="aips", bufs=2, space="PSUM"))
        mm_pool = ctx.enter_context(tc.tile_pool(name="mmps", bufs=2, space="PSUM"))
        fl_pool = ctx.enter_context(tc.tile_pool(name="fl", bufs=1))

        whj_sb = const_pool.tile([IN_CH, 2 * HC], BF16)
        nc.sync.dma_start(whj_sb[:], whj_d[:])
        wi_sb = const_pool.tile([IN_CH, HC], BF16)
        nc.sync.dma_start(wi_sb[:], wi_d[:])
        mask_sb = const_pool.tile([128, 16], I16)
        nc.sync.dma_start(mask_sb[:], mask_d[:])
        iota_sb = const_pool.tile([128, 128], BF16)
        nc.sync.dma_start(iota_sb[:], iota_d[:])
        bias_sb = const_pool.tile([128, NPC * HC], F32)
        nc.sync.dma_start(bias_sb[:], bias_d[:])

        ai_sb = ai_pool.tile([128, NPC * HC], BF16)

        # chunks of <=8 bins: each ai PSUM chunk stays within one 2KB bank
        chunks = [(i, min(i + 8, NB)) for i in range(0, NB, 8)]

        def emit_once(rep):
            # ---- Phase A1: att_i -> SBUF (tiny, first so pair 0 can start)
            for t in range(NPC if do_build else 0):
                nf2 = b_in.tile([128, 128], BF16, tag="nf2", name="nf2")
                dmae = nc.scalar if t % 2 == 0 else nc.sync
                dmae.dma_start(nf2[:], nodes_ftT[:, 128 * t:128 * (t + 1)])
                ps2 = b_ps.tile([128, 2, 2 * HC], F32, tag="bps", name="bps2")
                nc.tensor.matmul(ps2[:, 0, 0:HC], nf2[:], wi_sb[:],
                                 start=True, stop=True)
                if t % 2 == 0:
                    nc.scalar.copy(ai_sb[:, HC * t:HC * (t + 1)], ps2[:, 0, 0:HC])
                else:
                    nc.vector.tensor_copy(ai_sb[:, HC * t:HC * (t + 1)],
                                          ps2[:, 0, 0:HC])

            # ---- Phase A2: hj tables (lo then hi; batched DMAs, alternating
            # HWDGE engines)
            for t2 in range(T_TILES // 2 if do_build else 0):
                t = 2 * t2
                dmae = nc.sync if t2 % 2 == 0 else nc.scalar
                nf = b_in.tile([128, 2, 128], BF16, tag="nf", name="nf")
                dmae.dma_start(
                    nf[:].rearrange("p a b -> p (a b)"),
                    nodes_ftT[:, 128 * t:128 * (t + 2)])
                ps = b_ps.tile([128, 2, 2 * HC], F32, tag="bps", name="bps")
                nc.tensor.matmul(ps[:, 0, :], nf[:, 0, :], whj_sb[:],
                                 start=True, stop=False)
                nc.tensor.matmul(ps[:, 1, :], nf[:, 1, :], whj_sb[:],
                                 start=False, stop=True)
                st = b_st.tile([128, 2, 2 * HC], BF16, tag="bst", name="bst")
                if t2 % 2 == 0:
                    nc.vector.tensor_copy(st[:], ps[:])
                else:
                    nc.scalar.copy(st[:], ps[:])
                tab = hj_lo if t < HT else hj_hi
                rbase = 128 * t if t < HT else 128 * (t - HT)
                dmae.dma_start(
                    tab[rbase:rbase + 256, :].rearrange("(a p) b -> p a b", p=128),
                    st[:])

            # ---- Phase B: per destination pair, 4-stage software pipeline so
            # no engine queue blocks on a same-iteration cross-engine dep:
            #   produceA(p):  idx DMAs, G memset, gathers, OH/TMP/OHT (DVE)
            #   produceB(p):  ai matmuls (PE), aips->ai_pe copies (scalar)
            #   consume1(p):  s = ai+aj, leaky (DVE); x = exp (ACT)
            #   consume2(p):  y = h*x (DVE), scatter (PE), flush (scalar)
            stage_n = fl_pool.tile([128, NPC * HC], F32, tag="sn", name="sn")
            stage_d = fl_pool.tile([128, NPC * HC], F32, tag="sd", name="sd")

            tiles = {}

            def produceA(p):
                li = idx_pool.tile([128, K_LO * 8], I16, tag="li", name="li")
                nc.sync.dma_start(li[:], loidx_d[:, p * K_LO * 8:(p + 1) * K_LO * 8])
                hi_t = idx_pool.tile([128, K_HI * 8], I16, tag="hi", name="hi")
                nc.sync.dma_start(hi_t[:], hiidx_d[:, p * K_HI * 8:(p + 1) * K_HI * 8])
                tl_t = idx_pool.tile([128, NB], BF16, tag="tl", name="tl")
                nc.sync.dma_start(tl_t[:], tl_d[:, p * NB:(p + 1) * NB])
                bt = idx_pool.tile([128, NB * 8], I16, tag="bt", name="bt")
                nc.sync.dma_start(bt[:], bits_d[:, p * NB * 8:(p + 1) * NB * 8])

                G = g_pool.tile([128, NB, 2 * HC], BF16, tag="G", name="G")
                nc.vector.memset(G[:], 0.0)
                KL2 = (K_LO + 1) // 2
                KH2 = (K_HI + 1) // 2
                gcalls = [
                    (hj_lo, li, 0, KL2, 0),
                    (hj_lo, li, KL2, K_LO, 0),
                    (hj_hi, hi_t, 0, KH2, K_LO),
                    (hj_hi, hi_t, KH2, K_HI, K_LO),
                ]
                for k, (tab, it_, c0, c1, goff) in enumerate(gcalls):
                    nc.gpsimd.dma_gather(
                        out_ap=G[:, goff + c0:goff + c1, :], in_ap=tab[:],
                        idxs_ap=it_[:, c0 * 8:c1 * 8],
                        num_idxs=(c1 - c0) * 128,
                        num_idxs_reg=int(cnt_u[p, k]), elem_size=2 * HC,
                        queue_num=(p + k) % 4, single_packet=False)
                tiles[p] = {"G": G}
                if not do_dve:
                    return

                OH = oh_pool.tile([128, NB, 128], BF16, tag="OH", name="OH")
                nc.vector.tensor_tensor(
                    out=OH[:],
                    in0=tl_t[:].broadcast_to([128, NB, 128]),
                    in1=iota_sb[:].rearrange("p (a k) -> p a k", a=1)
                        .broadcast_to([128, NB, 128]),
                    op=ALU.is_equal)
                TMP = tmp_pool.tile([128, NB * 8, 16], I16, tag="TMP", name="TMP")
                nc.vector.tensor_tensor(
                    out=TMP[:],
                    in0=bt[:].broadcast_to([128, NB * 8, 16]),
                    in1=mask_sb[:].rearrange("p (a j) -> p a j", a=1)
                        .broadcast_to([128, NB * 8, 16]),
                    op=ALU.bitwise_and)
                OHT = oht_pool.tile([128, NB * 8, 16], BF16, tag="OHT", name="OHT")
                nc.vector.tensor_tensor(
                    out=OHT[:],
                    in0=TMP[:],
                    in1=mask_sb[:].rearrange("p (a j) -> p a j", a=1)
                        .broadcast_to([128, NB * 8, 16]),
                    op=ALU.is_equal)
                tiles[p]["OH"] = OH
                tiles[p]["OHT"] = OHT

            def produceB(p):
                OHT2 = tiles[p]["OHT"][:].rearrange("p a j -> p (a j)")
                AIPE = aipe_pool.tile([128, NB, HC], BF16, tag="AIPE", name="AIPE")
                for ci, (b0, b1) in enumerate(chunks):
                    aips = aips_pool.tile([128, 8 * HC], F32,
                                          tag="aip", name=f"aip{ci}")
                    for bl in range(b0, b1):
                        nc.tensor.matmul(
                            aips[:, (bl - b0) * HC:(bl - b0 + 1) * HC],
                            OHT2[:, bl * 128:(bl + 1) * 128],
                            ai_sb[:, p * HC:(p + 1) * HC],
                            start=(bl == b0), stop=(bl == b1 - 1))
                    nc.scalar.copy(
                        AIPE[:, b0:b1, :],
                        aips[:, 0:(b1 - b0) * HC].rearrange("p (a c) -> p a c", c=HC))
                tiles[p]["AIPE"] = AIPE

            def consume1(p):
                G = tiles[p]["G"]
                AIPE = tiles[p].pop("AIPE")
                S = s_pool.tile([128, NB, HC], BF16, tag="S", name="S")
                # s = att_i + att_j
                nc.vector.tensor_tensor(
                    out=S[:], in0=AIPE[:], in1=G[:, :, HC:2 * HC], op=ALU.add)
                # l = max(0.2*s, s)
                S2 = S[:].rearrange("p a c -> p (a c)")
                nc.vector.scalar_tensor_tensor(
                    out=S2, in0=S2, scalar=NEG_SLOPE, in1=S2,
                    op0=ALU.mult, op1=ALU.max)
                # x = exp(l) -> att_j half of G
                nc.scalar.activation(G[:, :, HC:2 * HC], S[:], ACT.Exp)

            def consume2(p):
                t = tiles.pop(p)
                G, OH = t["G"], t["OH"]
                # y = h * x -> h half of G
                nc.vector.tensor_tensor(
                    out=G[:, :, 0:HC], in0=G[:, :, 0:HC],
                    in1=G[:, :, HC:2 * HC], op=ALU.mult)
                MM = mm_pool.tile([128, 2 * HC], F32, tag="MM", name="MM")
                for bl in range(NB):
                    nc.tensor.matmul(
                        MM[:], OH[:, bl, :], G[:, bl, :],
                        start=(bl == 0), stop=(bl == NB - 1))
                nc.scalar.copy(stage_n[:, HC * p:HC * (p + 1)], MM[:, 0:HC])
                nc.scalar.copy(stage_d[:, HC * p:HC * (p + 1)], MM[:, HC:2 * HC])

            for it in range(NPC + 4 if do_gather else 0):
                if do_dve and do_mm and 0 <= it - 4:
                    consume2(it - 4)
                if it < NPC:
                    produceA(it)
                if do_dve and 0 <= it - 1 < NPC:
                    produceB(it - 1)
                if do_dve and 0 <= it - 3 < NPC:
                    consume1(it - 3)

            # ---- Phase C: out = numer / (denom + eps) + bias
            if not do_mm:
                nc.vector.memset(stage_n[:], 0.0)
                nc.vector.memset(stage_d[:], 1.0)
            nc.vector.tensor_scalar_add(stage_d[:], stage_d[:], 1e-16)
            lnd = fl_pool.tile([128, NPC * HC], F32, tag="lnd", name="lnd")
            nc.scalar.activation(lnd[:], stage_d[:], ACT.Ln)
            nc.scalar.activation(lnd[:], lnd[:], ACT.Exp, scale=-1.0)
            nc.vector.tensor_tensor(out=stage_n[:], in0=stage_n[:], in1=lnd[:],
                                    op=ALU.mult)
            nc.vector.tensor_tensor(out=stage_n[:], in0=stage_n[:], in1=bias_sb[:],
                                    op=ALU.add)

            out_view = out_d[:].rearrange("(pr p) c -> p pr c", p=128)
            st_view = stage_n[:].rearrange("p (pr c) -> p pr c", c=HC)
            nc.sync.dma_start(out_view, st_view)

        for rep in range(repeat):
            emit_once(rep)
            if repeat > 1:
                tc.strict_bb_all_engine_barrier()

    nc.compile()
    return nc


def kernel(**inputs):
    cfg = _cfg()
    in_maps, meta = _prep(inputs, cfg)
    nc = _build_program(cfg, meta["K_LO"], meta["K_HI"], meta["cnt_u"])

    from concourse import bass_utils
    res = bass_utils.run_bass_kernel_spmd(
        nc, in_maps, core_ids=list(range(cfg["NC"])),
        trace=bool(int(os.environ.get("GAT_TRACE", "0"))),
    )
    kernel.last_result = res
    kernel.last_ctx = (nc, in_maps, cfg)

    NSHARD = cfg["NSHARD"]
    out_full = np.zeros((cfg["NC"] * NSHARD, HC), dtype=np.float32)
    for c in range(cfg["NC"]):
        out_full[c * NSHARD:(c + 1) * NSHARD] = res.results[c]["out"]
    return out_full[:cfg["N"]]
